# revision 1
# baseline (speedup 1.0000x reference)
"""Distributed Trainium2 Bass kernel for the GNN message-passing problem.

Sharding: edges sorted by dst; nodes partitioned into 8 contiguous ranges
(2500 real -> 2560 slots = 20 tiles of 128 per core).  Each core owns all
edges into its node range, so segment aggregations are core-local:
  - segment-sum (GCN) = one-hot matmul per 128-node tile, PSUM-accumulated
  - segment-max (EdgeConv) = padded dma_gather + DVE max tree
Per-edge gathers via dma_gather; hidden node features AllGather-ed between
layers; all MLP weights replicated.
"""

import os
import sys

import numpy as np

sys.path.insert(0, "/opt/trn_rl_repo")

import ml_dtypes  # noqa: E402

N, E = 20000, 640000
F, H, T = 64, 256, 64
CORES = 8
NPC = N // CORES          # 2500 real nodes per core
P = 128
NT = 20                   # node tiles per core
NSLOT = NT * P            # 2560 node slots per core
GSLOT = CORES * NSLOT     # 20480 global node slots
BF16 = ml_dtypes.bfloat16

LAST_RESULT = None


def _node_slot(node):
    return (node // NPC) * NSLOT + (node % NPC)


# ---------------------------------------------------------------------------
# host-side index/table preprocessing (index work + sharding only)
# ---------------------------------------------------------------------------
def _preprocess(x, edge_index, edge_attr):
    src = edge_index[0].astype(np.int64)
    dst = edge_index[1].astype(np.int64)
    deg_global = np.bincount(dst, minlength=N).astype(np.float32) + 1.0

    cores = []
    Cmax = 0
    Dmax_t = np.zeros(NT, dtype=np.int64)
    for k in range(CORES):
        lo, hi = k * NPC, (k + 1) * NPC
        sel = (dst >= lo) & (dst < hi)
        ls = src[sel]
        ld = dst[sel] - lo
        eid = np.nonzero(sel)[0]
        order = np.argsort(ld, kind="stable")
        ls, ld, eid = ls[order], ld[order], eid[order]
        cnt = np.bincount(ld // P, minlength=NT)
        Cmax = max(Cmax, int(np.ceil(cnt.max() / P)))
        degs = np.bincount(ld, minlength=NSLOT)
        for t in range(NT):
            Dmax_t[t] = max(Dmax_t[t], degs[t * P:(t + 1) * P].max())
        cores.append(dict(ls=ls, ld=ld, eid=eid, cnt=cnt, degs=degs))

    C = int(np.ceil(Cmax / 4.0) * 4)           # chunks per tile (mult of 4)
    SLOTS = NT * C * P
    D_t = [int(np.ceil(max(d, 1) / 8.0) * 8) for d in Dmax_t]
    TOTIDX = sum(P * d for d in D_t)

    x_b16 = np.zeros((N, 128), dtype=BF16)
    x_b16[:, :F] = x.astype(BF16)

    def idx16(a):
        # dma_gather layout: idx j at [j%16, j//16], replicated to all 8
        # 16-partition GPSIMD core groups
        n = a.shape[0]
        return np.ascontiguousarray(np.tile(a.reshape(n // 16, 16).T, (8, 1)))

    per_core = []
    for k in range(CORES):
        c = cores[k]
        ls, ld, eid, cnt, degs = c["ls"], c["ld"], c["eid"], c["cnt"], c["degs"]

        esrc_slot = np.zeros(SLOTS, dtype=np.int16)
        esrc_node = np.zeros(SLOTS, dtype=np.int16)
        edst_node = np.zeros(SLOTS, dtype=np.int16)
        edstloc = np.full(SLOTS, 999.0, dtype=np.float32)
        ea_slot = np.zeros((SLOTS, F), dtype=np.float32)
        tstart = np.zeros(NT + 1, dtype=np.int64)
        tstart[1:] = np.cumsum(cnt)
        for t in range(NT):
            s0, s1 = tstart[t], tstart[t + 1]
            base = t * C * P
            n = s1 - s0
            esrc_slot[base:base + n] = _node_slot(ls[s0:s1]).astype(np.int16)
            esrc_node[base:base + n] = ls[s0:s1].astype(np.int16)
            edst_node[base:base + n] = (ld[s0:s1] + k * NPC).astype(np.int16)
            edstloc[base:base + n] = (ld[s0:s1] - t * P).astype(np.float32)
            ea_slot[base:base + n] = edge_attr[eid[s0:s1]]

        maxidx = np.zeros(TOTIDX, dtype=np.int16)
        off = 0
        for t in range(NT):
            D = D_t[t]
            loc_deg = degs[t * P:(t + 1) * P]
            loc_start = np.zeros(P, dtype=np.int64)
            loc_start[1:] = np.cumsum(loc_deg)[:-1]
            blk = np.zeros((P, D), dtype=np.int64)
            for p in range(P):
                d0 = loc_deg[p]
                if d0 == 0:
                    continue                      # row 0, masked later
                r0 = loc_start[p]
                blk[p, :d0] = np.arange(r0, r0 + d0)
                blk[p, d0:] = r0                  # duplicate = max no-op
            maxidx[off:off + P * D] = blk.T.reshape(-1).astype(np.int16)
            off += P * D

        deg_loc = np.ones(NSLOT, dtype=np.float32)
        deg_loc[:NPC] = deg_global[k * NPC:(k + 1) * NPC]
        mask_loc = np.zeros(NSLOT, dtype=np.float32)
        mask_loc[:NPC] = (deg_global[k * NPC:(k + 1) * NPC] > 1.0)

        x_loc = np.zeros((NSLOT, F), dtype=np.float32)
        x_loc[:NPC] = x[k * NPC:(k + 1) * NPC]

        per_core.append(dict(
            esrc_slot=idx16(esrc_slot),
            esrc_node=idx16(esrc_node),
            edst_node=idx16(edst_node),
            maxidx=idx16(maxidx),
            edstloc=np.ascontiguousarray(edstloc.reshape(NT * C, P).T),
            eaT=np.ascontiguousarray(ea_slot.T).astype(BF16),
            degT=np.ascontiguousarray(deg_loc.reshape(NT, P).T),
            maskT=np.ascontiguousarray(mask_loc.reshape(NT, P).T),
            xT=np.ascontiguousarray(x_loc.T),
            x_nm=np.ascontiguousarray(
                x_loc.reshape(NT, P, F).transpose(1, 0, 2).reshape(P, NT * F)),
        ))

    return dict(C=C, SLOTS=SLOTS, D_t=D_t, TOTIDX=TOTIDX,
                x_b16=x_b16, per_core=per_core)


def _bcast(v):
    return np.ascontiguousarray(
        np.tile(np.asarray(v, np.float32)[None, :], (P, 1)))


def _kblocks(w, kb):
    """[kb*128, M] -> [128, kb*M] with block j at cols j*M:(j+1)*M"""
    w = np.asarray(w)
    K, M = w.shape
    return np.ascontiguousarray(
        w.reshape(kb, 128, M).transpose(1, 0, 2).reshape(128, kb * M))


def _prep_weights(w):
    eW1p = np.zeros((256, 256), dtype=np.float32)
    eW1p[0:64] = w["e_W1"][0:64]
    eW1p[128:192] = w["e_W1"][64:128]
    eW1p[192:256] = w["e_W1"][128:192]
    return dict(
        W1=np.ascontiguousarray(w["conv1_W"], dtype=np.float32),
        b1=_bcast(w["conv1_b"]),
        eW1p=_kblocks(eW1p, 2).astype(BF16),                 # [128, 2*256]
        eb1=_kblocks(np.asarray(w["e_b1"], np.float32)[:, None], 2),
        eW2=_kblocks(w["e_W2"].astype(np.float32), 2).astype(BF16),
        eb2=_bcast(w["e_b2"]),
        W2=_kblocks(w["conv2_W"], 4).astype(BF16),           # [128, 4*256]
        b2=_bcast(w["conv2_b"]),
        W3=_kblocks(w["conv3_W"], 2).astype(BF16),           # [128, 2*64]
        b3=_bcast(w["conv3_b"]),
        l1W=np.ascontiguousarray(w["lin1_W"], dtype=np.float32),
        l1b=np.ascontiguousarray(np.asarray(w["lin1_b"], np.float32)[:, None]),
        l2W=np.ascontiguousarray(w["lin2_W"], dtype=np.float32),
        l2b=np.ascontiguousarray(np.asarray(w["lin2_b"], np.float32)[:, None]),
        l3W=np.ascontiguousarray(w["lin3_W"], dtype=np.float32),
        l3b=np.ascontiguousarray(np.asarray(w["lin3_b"], np.float32)[:, None]),
        l4W=np.ascontiguousarray(w["lin4_W"], dtype=np.float32),
        l4b=np.ascontiguousarray(np.asarray(w["lin4_b"], np.float32)[:, None]),
    )


# ---------------------------------------------------------------------------
# pure-numpy emulation of the device dataflow (validates all index tables)
# ---------------------------------------------------------------------------
def _emulate(pre, wts, wraw):
    C, SLOTS, D_t = pre["C"], pre["SLOTS"], pre["D_t"]
    xb = pre["x_b16"].astype(np.float32)

    def unpack16(a, n):
        return np.ascontiguousarray(a.T).reshape(-1)[:n].astype(np.int64)

    def unkb(wb, kb, M):
        return wb.astype(np.float32).reshape(
            128, kb, M).transpose(1, 0, 2).reshape(kb * 128, M)

    dis_all, dis2_all = [], []
    for k in range(CORES):
        deg = pre["per_core"][k]["degT"].T.reshape(-1)
        dis2_all.append(1.0 / deg)
        dis_all.append(np.sqrt(1.0 / deg))

    def gcn(hws_full, width, elem):
        outs = []
        for k in range(CORES):
            pc = pre["per_core"][k]
            esrc = unpack16(pc["esrc_slot"], SLOTS)
            dl = pc["edstloc"].T.reshape(-1)
            g = hws_full.astype(np.float32)[esrc][:, :width]
            agg = np.zeros((NSLOT, width), dtype=np.float32)
            for t in range(NT):
                base = t * C * P
                for c in range(C):
                    sl = slice(base + c * P, base + (c + 1) * P)
                    S = (dl[sl][:, None] == np.arange(P)[None, :])
                    agg[t * P:(t + 1) * P] += S.astype(np.float32).T @ g[sl]
            outs.append(agg)
        return outs

    # GCN1
    hws1 = np.zeros((GSLOT, 256), dtype=BF16)
    hw1_all = []
    for k in range(CORES):
        hw1 = pre["per_core"][k]["xT"].T @ wts["W1"]
        hw1_all.append(hw1)
        hws1[k * NSLOT:(k + 1) * NSLOT] = (hw1 * dis_all[k][:, None]).astype(BF16)
    out1_all = []
    for k, agg in enumerate(gcn(hws1, 256, 256)):
        out1_all.append(np.maximum(
            agg * dis_all[k][:, None] + hw1_all[k] * dis2_all[k][:, None]
            + wraw["conv1_b"][None, :], 0.0).astype(BF16).astype(np.float32))

    # EdgeConv
    oute1_all = []
    eW1p = unkb(wts["eW1p"], 2, 256)
    eW2 = unkb(wts["eW2"], 2, 256)
    eb1 = unkb(wts["eb1"], 2, 1)[:, 0]
    for k in range(CORES):
        pc = pre["per_core"][k]
        xd = xb[unpack16(pc["edst_node"], SLOTS)]
        xs = xb[unpack16(pc["esrc_node"], SLOTS)]
        ea = pc["eaT"].T.astype(np.float32)
        catB = np.concatenate([xs[:, :64], ea], 1)
        m1 = np.maximum(xd @ eW1p[0:128] + catB @ eW1p[128:256]
                        + eb1[None, :], 0.0).astype(BF16).astype(np.float32)
        m2 = (m1 @ eW2).astype(BF16)
        maxidx = unpack16(pc["maxidx"], pre["TOTIDX"])
        oute1 = np.zeros((NSLOT, 256), dtype=np.float32)
        off = 0
        for t in range(NT):
            D = D_t[t]
            idx = maxidx[off:off + P * D].reshape(D, P)
            acc = m2[t * C * P + idx[0]].astype(np.float32)
            for d in range(1, D):
                acc = np.maximum(acc, m2[t * C * P + idx[d]].astype(np.float32))
            v = np.maximum(acc + wraw["e_b2"][None, :], 0.0)
            v *= pc["maskT"][:, t][:, None]
            oute1[t * P:(t + 1) * P] = v
            off += P * D
        oute1_all.append(oute1.astype(BF16).astype(np.float32))

    # GCN2
    W2 = unkb(wts["W2"], 4, 256)
    hws2 = np.zeros((GSLOT, 256), dtype=BF16)
    hw2_all = []
    for k in range(CORES):
        h2 = np.concatenate([out1_all[k], oute1_all[k]], 1).astype(BF16)
        hw2 = h2.astype(np.float32) @ W2
        hw2_all.append(hw2)
        hws2[k * NSLOT:(k + 1) * NSLOT] = (hw2 * dis_all[k][:, None]).astype(BF16)
    out2_all = []
    for k, agg in enumerate(gcn(hws2, 256, 256)):
        out2_all.append(np.maximum(
            agg * dis_all[k][:, None] + hw2_all[k] * dis2_all[k][:, None]
            + wraw["conv2_b"][None, :], 0.0).astype(BF16).astype(np.float32))

    # GCN3
    W3 = unkb(wts["W3"], 2, 64)
    hws3 = np.zeros((GSLOT, 128), dtype=BF16)
    hw3_all = []
    for k in range(CORES):
        hw3 = out2_all[k].astype(BF16).astype(np.float32) @ W3
        hw3_all.append(hw3)
        hws3[k * NSLOT:(k + 1) * NSLOT, :64] = (
            hw3 * dis_all[k][:, None]).astype(BF16)
    out_rows = [np.zeros(N, np.float32), np.zeros(N, np.float32)]
    for k, agg in enumerate(gcn(hws3, 64, 128)):
        out3 = np.maximum(
            agg * dis_all[k][:, None] + hw3_all[k] * dis2_all[k][:, None]
            + wraw["conv3_b"][None, :], 0.0).astype(BF16).astype(np.float32)
        x_nm = pre["per_core"][k]["x_nm"].reshape(P, NT, F) \
            .transpose(1, 0, 2).reshape(NSLOT, F)
        hcat = np.concatenate([x_nm, out3], 1)

        def sp(v):
            return np.log1p(np.exp(-np.abs(v))) + np.maximum(v, 0.0)

        l1 = sp(hcat @ wts["l1W"] + wts["l1b"][:, 0][None, :])
        l2 = sp(l1 @ wts["l2W"] + wts["l2b"][:, 0][None, :])
        l3 = sp(l2 @ wts["l3W"] + wts["l3b"][:, 0][None, :])
        l4 = l3 @ wts["l4W"] + wts["l4b"][:, 0][None, :]
        out_rows[0][k * NPC:(k + 1) * NPC] = l4[:NPC, 0]
        out_rows[1][k * NPC:(k + 1) * NPC] = l4[:NPC, 1]
    return out_rows[0], out_rows[1]


# ---------------------------------------------------------------------------
# device kernel builder (single SPMD graph for all 8 cores)
# ---------------------------------------------------------------------------
def _build(C, SLOTS, D_t, TOTIDX):
    STAGES = int(os.environ.get("GNN_STAGES", "9"))
    SUB = int(os.environ.get("GNN_SUB", "9"))
    from concourse import mybir, bacc
    import concourse.tile as tile
    from concourse.masks import make_identity

    f32 = mybir.dt.float32
    b16 = mybir.dt.bfloat16
    i16 = mybir.dt.int32 if False else mybir.dt.int16
    AF = mybir.ActivationFunctionType
    OP = mybir.AluOpType
    HC = C // 2

    nc = bacc.Bacc(None)

    def inp(name, shape, dt=f32):
        return nc.declare_dram_parameter(name, list(shape), dt, isOutput=False)

    xT_e = inp("xT", (F, NSLOT))
    xnm_e = inp("x_nm", (P, NT * F))
    xb16_e = inp("x_b16", (N, 128), b16)
    eaT_e = inp("eaT", (F, SLOTS), b16)
    degT_e = inp("degT", (P, NT))
    maskT_e = inp("maskT", (P, NT))
    edstloc_e = inp("edstloc", (P, NT * C))
    esrc_slot_e = inp("esrc_slot", (128, SLOTS // 16), i16)
    esrc_node_e = inp("esrc_node", (128, SLOTS // 16), i16)
    edst_node_e = inp("edst_node", (128, SLOTS // 16), i16)
    maxidx_e = inp("maxidx", (128, TOTIDX // 16), i16)
    W1_e = inp("W1", (F, 256)); b1_e = inp("b1", (P, 256))
    eW1p_e = inp("eW1p", (P, 2 * 256), b16)
    eb1_e = inp("eb1", (P, 2))
    eW2_e = inp("eW2", (P, 2 * 256), b16)
    eb2_e = inp("eb2", (P, 256))
    W2_e = inp("W2", (P, 4 * 256), b16); b2_e = inp("b2", (P, 256))
    W3_e = inp("W3", (P, 2 * 64), b16); b3_e = inp("b3", (P, 64))
    l1W_e = inp("l1W", (P, 64)); l1b_e = inp("l1b", (64, 1))
    l2W_e = inp("l2W", (64, P)); l2b_e = inp("l2b", (P, 1))
    l3W_e = inp("l3W", (P, 32)); l3b_e = inp("l3b", (32, 1))
    l4W_e = inp("l4W", (32, 2)); l4b_e = inp("l4b", (2, 1))
    out_e = nc.declare_dram_parameter("out", [2, NSLOT], f32, isOutput=True)

    msg_d = nc.dram_tensor("msg_d", [NT * C * P, 256], b16)
    hws1_d = nc.dram_tensor("hws1_d", [NSLOT, 256], b16)
    hws2_d = nc.dram_tensor("hws2_d", [NSLOT, 256], b16)
    hws3_d = nc.dram_tensor("hws3_d", [NSLOT, 128], b16)
    hws1_f = nc.dram_tensor("hws1_f", [GSLOT, 256], b16, addr_space="Shared")
    hws2_f = nc.dram_tensor("hws2_f", [GSLOT, 256], b16, addr_space="Shared")
    hws3_f = nc.dram_tensor("hws3_f", [GSLOT, 128], b16, addr_space="Shared")

    RG = [list(range(CORES))]

    with tile.TileContext(nc) as tc:
        with (
            tc.tile_pool(name="const", bufs=1) as cpool,
            tc.tile_pool(name="big", bufs=1) as bigpool,
            tc.tile_pool(name="sb", bufs=2) as sb,
            tc.tile_pool(name="sb3", bufs=3) as sb3,
            tc.tile_pool(name="pagg", bufs=2, space="PSUM") as pagg,
            tc.tile_pool(name="pm1", bufs=2, space="PSUM") as pm1,
            tc.tile_pool(name="pmed", bufs=2, space="PSUM") as pmed,
            tc.tile_pool(name="ptp", bufs=2, space="PSUM") as ptp,
        ):
            def load(ext, shape, dt=f32, pool=None):
                nm = "c_" + ext.name
                t = (pool or cpool).tile(list(shape), dt, tag=nm, name=nm)
                nc.sync.dma_start(out=t[:], in_=ext[:])
                return t

            xT = load(xT_e, (F, NSLOT))
            x_nm = load(xnm_e, (P, NT * F))
            degT = load(degT_e, (P, NT))
            maskT = load(maskT_e, (P, NT))
            edstloc = load(edstloc_e, (P, NT * C))
            esrc_slot = load(esrc_slot_e, (128, SLOTS // 16), i16)
            W1 = load(W1_e, (F, 256)); b1 = load(b1_e, (P, 256))
            eW1p = load(eW1p_e, (P, 2 * 256), b16)
            eb1 = load(eb1_e, (P, 2))
            eW2 = load(eW2_e, (P, 2 * 256), b16)
            eb2 = load(eb2_e, (P, 256))
            W2 = load(W2_e, (P, 4 * 256), b16); b2 = load(b2_e, (P, 256))
            W3 = load(W3_e, (P, 2 * 64), b16); b3 = load(b3_e, (P, 64))
            l1W = load(l1W_e, (P, 64)); l1b = load(l1b_e, (64, 1))
            l2W = load(l2W_e, (64, P)); l2b = load(l2b_e, (P, 1))
            l3W = load(l3W_e, (P, 32)); l3b = load(l3b_e, (32, 1))
            l4W = load(l4W_e, (32, 2)); l4b = load(l4b_e, (2, 1))

            ident = cpool.tile([P, P], f32)
            make_identity(nc, ident[:])
            ident_b = cpool.tile([P, P], b16)
            nc.vector.tensor_copy(out=ident_b[:], in_=ident[:])
            iota_i = cpool.tile([P, P], mybir.dt.int32)
            nc.gpsimd.iota(iota_i[:], pattern=[[1, P]], base=0,
                           channel_multiplier=0)
            iota_f = cpool.tile([P, P], f32)
            nc.vector.tensor_copy(out=iota_f[:], in_=iota_i[:])

            dis2 = cpool.tile([P, NT], f32)
            nc.vector.reciprocal(out=dis2[:], in_=degT[:])
            dis = cpool.tile([P, NT], f32)
            nc.scalar.activation(dis[:], dis2[:], AF.Sqrt)

            out1_sb = bigpool.tile([P, NT * 256], b16)
            oute1_sb = bigpool.tile([P, NT * 256], b16)
            out2_sb = bigpool.tile([P, NT * 256], b16)
            out3_sb = bigpool.tile([P, NT * 64], b16)
            hw1_sb = bigpool.tile([P, NT * 256], b16)
            hw2_sb = bigpool.tile([P, NT * 256], b16)
            hw3_sb = bigpool.tile([P, NT * 64], b16)

            # ---------------- stage A: hw1 / hws1 / AllGather --------------
            for t in range(NT):
                ps = pmed.tile([P, 256], f32, tag="med")
                nc.tensor.matmul(ps[:], lhsT=xT[:, t * P:(t + 1) * P],
                                 rhs=W1[:], start=True, stop=True)
                nc.scalar.activation(hw1_sb[:, t * 256:(t + 1) * 256], ps[:],
                                     AF.Copy)
                hws = sb.tile([P, 256], b16, tag="hwsw")
                nc.vector.tensor_scalar(
                    out=hws[:], in0=ps[:], scalar1=dis[:, t:t + 1],
                    scalar2=None, op0=OP.mult)
                nc.sync.dma_start(
                    out=hws1_d[:].rearrange("(t p) h -> p t h", p=P)[:, t, :],
                    in_=hws[:])
            nc.gpsimd.collective_compute(
                "AllGather", OP.bypass, replica_groups=RG,
                ins=[hws1_d[:]], outs=[hws1_f[:]])

            # ---------------- GCN aggregation helper -----------------------
            def gcn_agg(hws_f, width, hw_sb, bias, out_sb, elem):
                nh = HC * P
                for t in range(NT if SUB >= 2 else 0):
                    ps = pagg.tile([P, width], f32, tag="agg")
                    for half in range(2):
                        g = sb.tile([P, HC * elem], b16, tag="gg")
                        col0 = (t * C * P + half * nh) // 16
                        nc.gpsimd.dma_gather(
                            g[:].rearrange("p (c h) -> p c h", h=elem),
                            hws_f[:], esrc_slot[:, col0:col0 + nh // 16],
                            nh, nh, elem, single_packet=False)
                        for cc in range(HC if SUB >= 3 else 0):
                            c = half * HC + cc
                            S = sb3.tile([P, P], b16, tag="S")
                            nc.vector.tensor_scalar(
                                out=S[:], in0=iota_f[:],
                                scalar1=edstloc[:, t * C + c:t * C + c + 1],
                                scalar2=None, op0=OP.is_equal)
                            nc.tensor.matmul(
                                ps[:], lhsT=S[:],
                                rhs=g[:, cc * elem:cc * elem + width],
                                start=(c == 0), stop=(c == C - 1))
                    if SUB < 3:
                        nc.vector.memset(ps[:], 0.0)
                    a = sb.tile([P, width], f32, tag="ep_a")
                    nc.vector.tensor_scalar(
                        out=a[:], in0=ps[:], scalar1=dis[:, t:t + 1],
                        scalar2=None, op0=OP.mult)
                    bsl = sb.tile([P, width], f32, tag="ep_b")
                    nc.vector.tensor_scalar(
                        out=bsl[:], in0=hw_sb[:, t * width:(t + 1) * width],
                        scalar1=dis2[:, t:t + 1], scalar2=None, op0=OP.mult)
                    nc.vector.tensor_tensor(out=a[:], in0=a[:], in1=bsl[:],
                                            op=OP.add)
                    nc.vector.tensor_tensor(out=a[:], in0=a[:],
                                            in1=bias[:, :width], op=OP.add)
                    nc.scalar.activation(
                        out_sb[:, t * width:(t + 1) * width], a[:], AF.Relu)

            gcn_agg(hws1_f, 256, hw1_sb, b1, out1_sb, 256)

            # ---------------- stage B: EdgeConv MLP ------------------------
            for t in range(NT if STAGES >= 2 else 0):
                for half in range(2):
                    ne = HC * P
                    base = t * C * P + half * ne
                    col0 = base // 16
                    A = sb.tile([P, ne], b16, tag="ecA")
                    Bt = sb.tile([P, ne], b16, tag="ecB")
                    idn = sb.tile([128, ne // 16], i16, tag="idn")
                    nc.sync.dma_start(out=idn[:],
                                      in_=edst_node_e[:, col0:col0 + ne // 16])
                    ids = sb.tile([128, ne // 16], i16, tag="ids")
                    nc.sync.dma_start(out=ids[:],
                                      in_=esrc_node_e[:, col0:col0 + ne // 16])
                    nc.gpsimd.dma_gather(
                        A[:].rearrange("p (a n) -> p a n", a=1), xb16_e[:],
                        idn[:], ne, ne, 128, transpose=True,
                        single_packet=False)
                    nc.gpsimd.dma_gather(
                        Bt[:].rearrange("p (a n) -> p a n", a=1), xb16_e[:],
                        ids[:], ne, ne, 128, transpose=True,
                        single_packet=False)
                    nc.sync.dma_start(out=Bt[64:128, :],
                                      in_=eaT_e[:, base:base + ne])
                    msg_sb = sb.tile([P, HC * 256], b16, tag="msg")
                    for s in range(HC // 2):
                        m1p = [pm1.tile([P, 256], f32, tag="m1",
                                        name=f"m1p{hh_}")
                               for hh_ in range(2)]
                        for hh in range(2):
                            nc.tensor.matmul(
                                m1p[hh][:],
                                lhsT=eW1p[:, hh * P:hh * P + P],
                                rhs=A[:, s * 256:(s + 1) * 256],
                                start=True, stop=False)
                            nc.tensor.matmul(
                                m1p[hh][:],
                                lhsT=eW1p[:, 256 + hh * P:256 + hh * P + P],
                                rhs=Bt[:, s * 256:(s + 1) * 256],
                                start=False, stop=True)
                        m1s = [sb.tile([P, 256], b16, tag="m1s",
                                       name=f"m1s{hh_}")
                               for hh_ in range(2)]
                        for hh in range(2):
                            nc.scalar.activation(
                                m1s[hh][:], m1p[hh][:], AF.Relu,
                                bias=eb1[:, hh:hh + 1])
                        for cc in range(2):
                            c = s * 2 + cc
                            m2p = pmed.tile([P, 256], f32, tag="med")
                            nc.tensor.matmul(
                                m2p[:], lhsT=m1s[0][:, cc * P:(cc + 1) * P],
                                rhs=eW2[:, 0:256], start=True, stop=False)
                            nc.tensor.matmul(
                                m2p[:], lhsT=m1s[1][:, cc * P:(cc + 1) * P],
                                rhs=eW2[:, 256:512], start=False, stop=True)
                            nc.scalar.activation(
                                msg_sb[:, c * 256:(c + 1) * 256], m2p[:],
                                AF.Copy)
                    nc.sync.dma_start(
                        out=msg_d[:].rearrange(
                            "(t h c p) w -> t h p c w",
                            t=NT, h=2, p=P)[t, half],
                        in_=msg_sb[:].rearrange("p (c w) -> p c w", w=256))

            # ---------------- stage C: EdgeConv max aggregation ------------
            moff = 0
            for t in range(NT if STAGES >= 3 else 0):
                D = D_t[t]
                acc = sb.tile([P, 256], b16, tag="macc")
                mxi = sb.tile([128, P * D // 16], i16, tag="mxi")
                nc.sync.dma_start(
                    out=mxi[:],
                    in_=maxidx_e[:, moff // 16:(moff + P * D) // 16])
                for g0 in range(D // 8):
                    gt = sb.tile([P, 8 * 256], b16, tag="mg")
                    nc.gpsimd.dma_gather(
                        gt[:].rearrange("p (c h) -> p c h", h=256),
                        msg_d[t * C * P:(t + 1) * C * P, :],
                        mxi[:, g0 * 8 * P // 16:(g0 + 1) * 8 * P // 16],
                        8 * P, 8 * P, 256, single_packet=False)
                    v = gt[:].rearrange("p (c h) -> p c h", h=256)
                    nc.vector.tensor_tensor(out=v[:, 0:4, :], in0=v[:, 0:4, :],
                                            in1=v[:, 4:8, :], op=OP.max)
                    nc.vector.tensor_tensor(out=v[:, 0:2, :], in0=v[:, 0:2, :],
                                            in1=v[:, 2:4, :], op=OP.max)
                    nc.vector.tensor_tensor(out=v[:, 0:1, :], in0=v[:, 0:1, :],
                                            in1=v[:, 1:2, :], op=OP.max)
                    if g0 == 0:
                        nc.vector.tensor_copy(out=acc[:], in_=gt[:, 0:256])
                    else:
                        nc.vector.tensor_tensor(out=acc[:], in0=acc[:],
                                                in1=gt[:, 0:256], op=OP.max)
                moff += P * D
                a = sb.tile([P, 256], f32, tag="ep_a")
                nc.vector.tensor_tensor(out=a[:], in0=acc[:], in1=eb2[:],
                                        op=OP.add)
                r = sb.tile([P, 256], f32, tag="ep_b")
                nc.scalar.activation(r[:], a[:], AF.Relu)
                nc.vector.tensor_scalar(
                    out=oute1_sb[:, t * 256:(t + 1) * 256], in0=r[:],
                    scalar1=maskT[:, t:t + 1], scalar2=None, op0=OP.mult)

            # ---------------- stage D: GCN2 --------------------------------
            for t in range(NT if STAGES >= 4 else 0):
                ps = pmed.tile([P, 256], f32, tag="med")
                for j in range(4):
                    srct = out1_sb if j < 2 else oute1_sb
                    jj = j % 2
                    tp = ptp.tile([P, P], b16, tag="tp")
                    nc.tensor.transpose(
                        tp[:],
                        srct[:, t * 256 + jj * P:t * 256 + (jj + 1) * P],
                        ident_b[:])
                    tps = sb.tile([P, P], b16, tag="h2T")
                    nc.vector.tensor_copy(out=tps[:], in_=tp[:])
                    nc.tensor.matmul(ps[:], lhsT=tps[:],
                                     rhs=W2[:, j * 256:(j + 1) * 256],
                                     start=(j == 0), stop=(j == 3))
                nc.scalar.activation(hw2_sb[:, t * 256:(t + 1) * 256], ps[:],
                                     AF.Copy)
                hws = sb.tile([P, 256], b16, tag="hwsw")
                nc.vector.tensor_scalar(
                    out=hws[:], in0=ps[:], scalar1=dis[:, t:t + 1],
                    scalar2=None, op0=OP.mult)
                nc.sync.dma_start(
                    out=hws2_d[:].rearrange("(t p) h -> p t h", p=P)[:, t, :],
                    in_=hws[:])
            if STAGES >= 4:
                nc.gpsimd.collective_compute(
                    "AllGather", OP.bypass, replica_groups=RG,
                    ins=[hws2_d[:]], outs=[hws2_f[:]])
                gcn_agg(hws2_f, 256, hw2_sb, b2, out2_sb, 256)

            # ---------------- stage E: GCN3 --------------------------------
            for t in range(NT if STAGES >= 5 else 0):
                ps = pmed.tile([P, 64], f32, tag="med")
                for j in range(2):
                    tp = ptp.tile([P, P], b16, tag="tp")
                    nc.tensor.transpose(
                        tp[:],
                        out2_sb[:, t * 256 + j * P:t * 256 + (j + 1) * P],
                        ident_b[:])
                    tps = sb.tile([P, P], b16, tag="h2T")
                    nc.vector.tensor_copy(out=tps[:], in_=tp[:])
                    nc.tensor.matmul(ps[:], lhsT=tps[:],
                                     rhs=W3[:, j * 64:(j + 1) * 64],
                                     start=(j == 0), stop=(j == 1))
                nc.scalar.activation(hw3_sb[:, t * 64:(t + 1) * 64], ps[:],
                                     AF.Copy)
                hws = sb.tile([P, 128], b16, tag="hwsw")
                nc.gpsimd.memset(hws[:], 0.0)
                nc.vector.tensor_scalar(
                    out=hws[:, 0:64], in0=ps[:], scalar1=dis[:, t:t + 1],
                    scalar2=None, op0=OP.mult)
                nc.sync.dma_start(
                    out=hws3_d[:].rearrange("(t p) h -> p t h", p=P)[:, t, :],
                    in_=hws[:])
            if STAGES >= 5:
                nc.gpsimd.collective_compute(
                    "AllGather", OP.bypass, replica_groups=RG,
                    ins=[hws3_d[:]], outs=[hws3_f[:]])
                gcn_agg(hws3_f, 64, hw3_sb, b3, out3_sb, 128)

            # ---------------- stage F: head MLP (feature-major) ------------
            def softplus(dst_ap, psum_ap, bias_col, rows):
                # softplus(z) = relu(z) + ln(1 + exp(-|z|)), z = psum + bias
                r = sb.tile([rows, P], f32, tag="spr", name="spr")
                nc.scalar.activation(r[:], psum_ap, AF.Relu, bias=bias_col)
                aa = sb.tile([rows, P], f32, tag="spa", name="spa")
                nc.scalar.activation(aa[:], psum_ap, AF.Abs, bias=bias_col)
                nc.scalar.activation(aa[:], aa[:], AF.Exp, scale=-1.0)
                nc.scalar.activation(aa[:], aa[:], AF.Ln, bias=1.0)
                nc.vector.tensor_tensor(out=dst_ap, in0=r[:], in1=aa[:],
                                        op=OP.add)

            for t in range(NT if STAGES >= 6 else 0):
                hcat = sb.tile([P, P], f32, tag="hcat")
                nc.vector.tensor_copy(
                    out=hcat[:, 0:64], in_=x_nm[:, t * 64:(t + 1) * 64])
                nc.vector.tensor_copy(
                    out=hcat[:, 64:128], in_=out3_sb[:, t * 64:(t + 1) * 64])
                tp = ptp.tile([P, P], f32, tag="tp")
                nc.tensor.transpose(tp[:], hcat[:], ident[:])
                hT = sb.tile([P, P], f32, tag="hT")
                nc.vector.tensor_copy(out=hT[:], in_=tp[:])
                p1 = pmed.tile([P, P], f32, tag="med")
                nc.tensor.matmul(p1[0:64, :], lhsT=l1W[:, 0:64], rhs=hT[:],
                                 start=True, stop=True)
                s1 = sb.tile([64, P], f32, tag="hl1")
                softplus(s1[:], p1[0:64, :], l1b[:], 64)
                p2 = pmed.tile([P, P], f32, tag="med")
                nc.tensor.matmul(p2[:], lhsT=l2W[:], rhs=s1[:],
                                 start=True, stop=True)
                s2 = sb.tile([P, P], f32, tag="hl2")
                softplus(s2[:], p2[:], l2b[:], P)
                p3 = pmed.tile([P, P], f32, tag="med")
                nc.tensor.matmul(p3[0:32, :], lhsT=l3W[:, 0:32], rhs=s2[:],
                                 start=True, stop=True)
                s3 = sb.tile([32, P], f32, tag="hl3")
                softplus(s3[:], p3[0:32, :], l3b[:], 32)
                p4 = pmed.tile([P, P], f32, tag="med")
                nc.tensor.matmul(p4[0:2, :], lhsT=l4W[:, 0:2], rhs=s3[:],
                                 start=True, stop=True)
                s4 = sb.tile([2, P], f32, tag="hl4")
                nc.scalar.activation(s4[:], p4[0:2, :], AF.Identity,
                                     bias=l4b[:])
                nc.sync.dma_start(out=out_e[:, t * P:(t + 1) * P], in_=s4[:])
            if STAGES < 6:
                z = sb.tile([2, NSLOT], f32, tag="zz")
                nc.vector.tensor_copy(out=z[:], in_=xT[0:2, 0:NSLOT])
                nc.sync.dma_start(out=out_e[:], in_=z[:])

    return nc


_CACHE = {}


def kernel(**inputs):
    global LAST_RESULT
    pre = _preprocess(inputs["x"], inputs["edge_index"], inputs["edge_attr"])
    wts = _prep_weights(inputs)

    if os.environ.get("GNN_EMULATE"):
        return _emulate(pre, wts, inputs)

    from concourse.bass_utils import run_bass_kernel_spmd

    nc = _build(pre["C"], pre["SLOTS"], pre["D_t"], pre["TOTIDX"])
    nc.finalize()

    in_maps = []
    for k in range(CORES):
        pc = pre["per_core"][k]
        m = dict(
            xT=pc["xT"], x_nm=pc["x_nm"], x_b16=pre["x_b16"],
            eaT=pc["eaT"], degT=pc["degT"], maskT=pc["maskT"],
            edstloc=pc["edstloc"], esrc_slot=pc["esrc_slot"],
            esrc_node=pc["esrc_node"], edst_node=pc["edst_node"],
            maxidx=pc["maxidx"],
        )
        m.update({k2: np.asarray(v) for k2, v in wts.items()})
        in_maps.append(m)

    res = run_bass_kernel_spmd(nc, in_maps, list(range(CORES)))
    LAST_RESULT = res
    _CACHE["nc"] = nc
    _CACHE["in_maps"] = in_maps
    r0 = np.concatenate(
        [res.results[k]["out"][0, :NPC] for k in range(CORES)])
    r1 = np.concatenate(
        [res.results[k]["out"][1, :NPC] for k in range(CORES)])
    return r0.astype(np.float32), r1.astype(np.float32)


def bench(n=3):
    """Re-run the cached compiled kernel; returns best wall seconds."""
    import time

    from concourse.bass_utils import run_bass_kernel_spmd

    nc, in_maps = _CACHE["nc"], _CACHE["in_maps"]
    best = float("inf")
    for _ in range(n):
        t0 = time.time()
        run_bass_kernel_spmd(nc, in_maps, list(range(CORES)))
        best = min(best, time.time() - t0)
    return best

